# revision 21
# baseline (speedup 1.0000x reference)
"""LoRA linear kernel for Trainium2 (Bass/Tile), 8-core SPMD.

Computes out = x @ (A @ B) * (alpha/r) for
  x: [4, 4096, 4096] f32, A: [4096, 16] f32, B: [16, 4096] f32
with alpha/r == 1.0.

Algorithm: reassociate as out = (x @ A) @ B  -- 128x fewer FLOPs than
materializing the 4096x4096 delta-weight.  Data-parallel over rows of x:
each of the 8 cores gets 2048 rows.

The kernel is HBM-bandwidth bound (~350-420 GB/s per core sustained),
so all large tensors move as bf16 (rel err ~4e-3, well under the 2e-2
gate):
  - x is cast to bf16 and pre-transposed/tiled per shard on the host, so
    the device streams xT straight into matmul1 (no on-device transpose)
    with every input DMA a single fully-contiguous 1 MiB block.
  - the output is produced as bf16 on device and upcast on the host.
HBM traffic per core: 16 MiB in + 16 MiB out (vs 64 MiB all-f32).

The m-rows are processed in two halves with software pipelining: while
half 0 streams out (phase 2: t @ B, PSUM->SBUF bf16 copies, out DMAs),
half 1 streams in (phase 1: xT chunks, tT accumulation in PSUM).  The
input stream is front-loaded (2-segment lead, then 1:1 with m-tiles).
All DMAs ride the sync HWDGE ring in emission order.

The PE mostly sits at the cold 1.2 GHz HAM clock (DMA pacing leaves
idle windows, so it keeps re-throttling).  Untiled, the two matmul
passes cost ~109 us at 1.2 GHz -- above the DMA floor.  Both matmuls
badly underuse the 128x128 array (mm1: lhs free dim 16; mm2: contract
dim 16), so each runs as TWO concurrent array tiles (tile_position):
  - mm1: 128x64 column tiling.  Lane 0 (array cols 0-63) accumulates
    the FULL k-sum for m-columns [0, 512) of the half, lane 1 (cols
    64-127) for m-columns [512, 1024); partials land at PSUM partitions
    0-15 / 64-79.  Splitting by m-range (not k-parity) means there is
    NO cross-lane reduction: the half boundary needs only 4 independent
    cast-copies (2 ACT + 2 DVE, pairwise parallel), so the out stream
    never stalls waiting on a serial reduce chain.
  - mm2: 64x128 row tiling; tT/B replicated at partitions 0-15 / 64-79,
    lane 0 takes n-chunks 0-3, lane 1 chunks 4-7.
This halves PE stream time (~56 us cold, ~28 us warm) so the PE never
paces the DMA streams.

Phase-2 PSUM is evacuated in [128, 1024] two-bank copies (one ACT + one
DVE per n-half), halving copy instruction count so the copy cadence
(~2.2-2.5 us per engine per m-tile) stays under the 1 MiB out-DMA time.

Input tiling: xt_pre[h*8+s] = [128, 4096] where column cc*1024+j of
partition p holds xT[(4s+cc)*128 + p, h*1024 + j]; one segment = 4
k-chunks of one m-half = 1 MiB contiguous.
"""

import os
import sys

import numpy as np

for _p in ("/opt/trn_rl_repo",):
    if os.path.isdir(_p) and _p not in sys.path:
        sys.path.insert(0, _p)

import concourse.bacc as bacc
import concourse.mybir as mybir
from concourse import tile
from concourse.bass_utils import run_bass_kernel_spmd

import ml_dtypes

R = 16
B_DIM = 4
SEQ = 4096
K = 4096  # in_features
N = 4096  # out_features
M_FULL = B_DIM * SEQ  # 16384
NCORES = 8
M_SHARD = M_FULL // NCORES  # 2048
SCALING = 16.0 / 16.0  # alpha / r == 1.0

KC = 128  # contraction chunk (partition dim of xT tiles)
N_KC = K // KC  # 32
MH = M_SHARD // 2  # 1024, m-half
ML = MH // 2  # 512, m-range per mm1 lane
CC_SEG = 4  # k-chunks per input segment
N_SEG = N_KC // CC_SEG  # 8 segments per half
MT = 128  # rows per m-tile in phase 2
N_MT_H = MH // MT  # 8 m-tiles per half
N_CHUNK = 512  # matmul2 moving free dim
OPS_W = 1024  # phase-2 psum tile width (2 banks), one copy each
LANE_P = 64  # partition offset of array lane 1 (2x tiling)

_F32 = mybir.dt.float32
_BF16 = mybir.dt.bfloat16


def _build_kernel(tc, nc, xt, a_pre, b_in, out):
    with (
        tc.tile_pool(name="const", bufs=1) as cpool,
        tc.tile_pool(name="xin", bufs=5) as xpool,
        tc.tile_pool(name="tps", bufs=2, space="PSUM") as tpsum,
        tc.tile_pool(name="tsb", bufs=2) as tspool,
        tc.tile_pool(name="ops", bufs=3, space="PSUM") as opsum,
        tc.tile_pool(name="osb", bufs=4) as opool,
    ):
        a_sb = cpool.tile([128, N_KC * R], _BF16, name="a_sb")
        b_sb = cpool.tile([LANE_P + R, N], _BF16, name="b_sb")

        def phase1_segment(h, s, tps):
            """DMA one 1 MiB input segment; 8 accumulating matmuls on two
            column-tiled PE lanes (lane 0: m [0,512), lane 1: m [512,1024),
            each over ALL k-chunks)."""
            xtile = xpool.tile([128, CC_SEG * MH], _BF16, tag="xt")
            nc.sync.dma_start(out=xtile, in_=xt[h * N_SEG + s : h * N_SEG + s + 1, :, :])
            for cc in range(CC_SEG):
                c = s * CC_SEG + cc
                for lane in range(2):
                    p0 = lane * LANE_P
                    nc.tensor.matmul(
                        tps[p0 : p0 + R, :],
                        a_sb[:, c * R : (c + 1) * R],
                        xtile[:, cc * MH + lane * ML : cc * MH + (lane + 1) * ML],
                        start=(c == 0),
                        stop=(c == N_KC - 1),
                        tile_position=(0, p0),
                    )

        def phase2_mtile(h, mt, ts):
            """out[m-tile, :] = ts[:, m-tile].T @ B on two row-tiled PE
            lanes; one [128, 1024] ACT + one DVE copy per n-half; 1 MiB
            out DMA."""
            osb = opool.tile([MT, N], _BF16)
            for nh in range(2):
                ops_a = opsum.tile([MT, OPS_W], _F32, tag="ops", name="ops_a")
                ops_b = opsum.tile([MT, OPS_W], _F32, tag="ops", name="ops_b")
                for q in range(2):
                    for lane, ops in ((0, ops_a), (1, ops_b)):
                        j = lane * 4 + nh * 2 + q
                        p0 = lane * LANE_P
                        nc.tensor.matmul(
                            ops[:, q * N_CHUNK : (q + 1) * N_CHUNK],
                            ts[p0 : p0 + R, mt * MT : (mt + 1) * MT],
                            b_sb[p0 : p0 + R, j * N_CHUNK : (j + 1) * N_CHUNK],
                            start=True,
                            stop=True,
                            tile_position=(p0, 0),
                        )
                nc.scalar.copy(osb[:, nh * OPS_W : (nh + 1) * OPS_W], ops_a[:])
                dve_dst = osb[:, 2048 + nh * OPS_W : 2048 + (nh + 1) * OPS_W]
                nc.vector.tensor_copy(dve_dst, ops_b[:])
            # Two half-row out-DMAs, each gated only on its own engine's
            # copies (ACT: n 0:2048, DVE: n 2048:4096) -- no cross-engine
            # last-copy tail in front of the DMA.
            row0 = h * MH + mt * MT
            nc.sync.dma_start(
                out=out[row0 : row0 + MT, 0 : N // 2], in_=osb[:, 0 : N // 2]
            )
            nc.sync.dma_start(
                out=out[row0 : row0 + MT, N // 2 : N], in_=osb[:, N // 2 : N]
            )

        def build_ts(tps):
            """Evacuate the two full-k lane partials to bf16 tT, replicated
            at partitions 0-15 and 64-79 for the mm2 row lanes.  Four
            independent copies: ACT and DVE each do one PSUM read and one
            SBUF replicate -- no serial cross-engine chain."""
            ts = tspool.tile([LANE_P + R, MH], _BF16, tag="ts", name="ts")
            nc.scalar.copy(ts[0:R, 0:ML], tps[0:R, :])
            nc.vector.tensor_copy(ts[LANE_P : LANE_P + R, ML:MH], tps[LANE_P : LANE_P + R, :])
            nc.scalar.copy(ts[LANE_P : LANE_P + R, 0:ML], ts[0:R, 0:ML])
            nc.vector.tensor_copy(ts[0:R, ML:MH], ts[LANE_P : LANE_P + R, ML:MH])
            return ts

        # Prologue: phase 1 of half 0.  The tiny A load goes first, then
        # the first x segment (critical path); the B load (only needed by
        # phase 2) follows it.
        nc.sync.dma_start(out=a_sb, in_=a_pre)
        tps0 = tpsum.tile([LANE_P + R, ML], _F32, tag="tps", name="tps0")
        phase1_segment(0, 0, tps0)
        nc.sync.dma_start(out=b_sb, in_=b_in)
        for s in range(1, N_SEG):
            phase1_segment(0, s, tps0)
        ts0 = build_ts(tps0)

        # Steady: half 1 phase 1 interleaved with half 0 phase 2, input
        # stream front-loaded (2-segment lead, then 1:1).  The half-1
        # tT evacuation is emitted right after the last input segment so
        # the remaining out-work hides its (short) latency.
        tps1 = tpsum.tile([LANE_P + R, ML], _F32, tag="tps", name="tps1")
        for s0 in range(3):
            phase1_segment(1, s0, tps1)
        ts1 = None
        for s in range(N_SEG):
            phase2_mtile(0, s, ts0)
            if s + 3 < N_SEG:
                phase1_segment(1, s + 3, tps1)
                if s + 3 == N_SEG - 1:
                    ts1 = build_ts(tps1)

        # Epilogue: phase 2 of half 1.
        for mt in range(N_MT_H):
            phase2_mtile(1, mt, ts1)


_NC_CACHE = None


def _get_nc():
    global _NC_CACHE
    if _NC_CACHE is not None:
        return _NC_CACHE
    nc = bacc.Bacc("TRN2", target_bir_lowering=False, debug=False)
    xt = nc.dram_tensor(
        "xt", [2 * N_SEG, 128, CC_SEG * MH], _BF16, kind="ExternalInput"
    ).ap()
    a_pre = nc.dram_tensor("a_pre", [128, N_KC * R], _BF16, kind="ExternalInput").ap()
    b_in = nc.dram_tensor("b_in", [LANE_P + R, N], _BF16, kind="ExternalInput").ap()
    out = nc.dram_tensor("out", [M_SHARD, N], _BF16, kind="ExternalOutput").ap()
    with tile.TileContext(nc) as tc:
        _build_kernel(tc, nc, xt, a_pre, b_in, out)
    nc.compile()
    _NC_CACHE = nc
    return nc


LAST_RESULTS = None


def kernel(x: np.ndarray, A: np.ndarray, B: np.ndarray) -> np.ndarray:
    global LAST_RESULTS
    assert x.shape == (B_DIM, SEQ, K), x.shape
    assert A.shape == (K, R), A.shape
    assert B.shape == (R, N), B.shape

    x_bf = np.asarray(x, dtype=np.float32).reshape(M_FULL, K).astype(ml_dtypes.bfloat16)
    a_np = np.asarray(A, dtype=np.float32)
    b_bf = (np.asarray(B, dtype=np.float32) * SCALING).astype(ml_dtypes.bfloat16)
    b_np = np.zeros((LANE_P + R, N), dtype=ml_dtypes.bfloat16)
    b_np[0:R] = b_bf
    b_np[LANE_P : LANE_P + R] = b_bf

    # Host pre-arrangement of A: [K, R] -> [128, (K/128) * R] bf16
    a_pre = np.ascontiguousarray(
        a_np.reshape(K // KC, KC, R).transpose(1, 0, 2).reshape(128, N_KC * R)
    ).astype(ml_dtypes.bfloat16)

    in_maps = []
    for i in range(NCORES):
        xT = x_bf[i * M_SHARD : (i + 1) * M_SHARD, :].T  # [K, M_SHARD]
        # [K, M] -> [s, cc, p, h, j] -> [h, s, p, cc, j] -> [16, 128, 4096]
        xt_i = np.ascontiguousarray(
            xT.reshape(N_SEG, CC_SEG, KC, 2, MH)
            .transpose(3, 0, 2, 1, 4)
            .reshape(2 * N_SEG, 128, CC_SEG * MH)
        )
        in_maps.append({"xt": xt_i, "a_pre": a_pre, "b_in": b_np})

    nc = _get_nc()
    trace = os.environ.get("KERNEL_TRACE", "0") == "1"
    tmpdir = os.environ.get("KERNEL_TMPDIR") or None
    res = run_bass_kernel_spmd(
        nc, in_maps, core_ids=list(range(NCORES)), trace=trace, tmpdir=tmpdir
    )
    LAST_RESULTS = res
    out = np.concatenate(
        [np.asarray(res.results[i]["out"], dtype=np.float32) for i in range(NCORES)],
        axis=0,
    )
    return out.reshape(B_DIM, SEQ, N)
